# revision 37
# baseline (speedup 1.0000x reference)
"""FP8 per-tensor dynamic-quantized Linear on 8 TRN2 NeuronCores.

Computes reference semantics of:
    x2 = x.reshape(-1, 4096)
    x_fp8, s_i = quantize_e4m3fn(x2)      # per-tensor amax -> scale
    w_fp8, s_w = quantize_e4m3fn(weight)
    out = (x_fp8.f32 @ w_fp8.f32.T) * (s_i * s_w) + bias

Sharding: token-parallel. Each core owns 1024 tokens of x (stored k-major,
DoubleRow pair-interleaved), the full weight (blocked layout), and a distinct
1/8 slice of the weight rows for the distributed amax pass. Global per-tensor
amax for x and w = local absmax reduce + one 8-core AllGather(max) of 2
floats.

TRN fp8_e4m3 saturates at +-240 (OCP e4m3fn goes to 448), so the device
quantizes with multiplier 224/amax == (448/amax)/2 exactly (power-of-two =>
bit-identical RNE mantissa rounding vs the reference) and the matmul output
is rescaled by (amax_x*amax_w)/50176 = 4*s_i*s_w to compensate the halvings.

Matmul runs in fp8 DoubleRow mode: stationary W tile [128,2,128] plane-major,
moving x tile pair-interleaved ([p, t, 2] memory viewed as [p, 2, t]) which
the PE streams at 2 fp8/cycle, accumulating 16 k-blocks into PSUM. Epilogue =
single ScalarE activation (scale + per-partition bias) emitting bf16; output
is written as contiguous [j, n, t] blocks and de-blocked (cast + transpose)
on the host. bf16 keeps the phase-C DMA demand (~2.2MB per 8.5us j-slot)
under the per-core HBM share so the PE never starves.

Scheduling notes (measured on the axon trn2 cores): the PE is power-throttled
to 81.25% duty (264ns per 512-row DR matmul; ~270us floor for the 1024
matmuls). Only SP and Activation have hardware DGE queues — gpsimd DMA is a
~80GB/s software queue and gpsimd tensor ops are software-emulated, so all
bulk DMA rides sync/scalar (W panels sync-only: the SP engine has no compute
duties, so the weight stream is never issued late) and all bulk elementwise
rides DVE/ACT. A warmup AllGather issued at t=0 absorbs the CC stack's
one-time setup; the real 8-byte amax AllGather costs ~18-25us. j=0/j=1
weight panels are prefetched during the collective window and their fp8
quantize is JIT-interleaved with the x8 quantize against the PE's
consumption order.
"""

import numpy as np

import concourse.bass as bass
import concourse.mybir as mybir
import concourse.tile as tile
from concourse import bacc, bass_isa, bass_utils

N_CORES = 8
XB, XS, K = 4, 2048, 4096   # x: [4, 2048, 4096]
N = 4096                    # weight: [N, K]
T = XB * XS                 # 8192 tokens
TC = T // N_CORES           # 1024 tokens per core
NB = N // 128               # 32 output-feature blocks
KB = K // 128               # 32 k subtiles of 128
KBB = K // 256              # 16 DoubleRow k blocks
TFREE = 512
TT = TC // TFREE            # 2 token tiles per core
WS = N // N_CORES           # 512 weight rows per core for amax

# x arrives as 8 2MB tiles (2 kbb each); each is absmax-reduced in halves,
# the last two (one per queue) in eighths so the final reduce trails the
# final DMA by ~0.6us
NXT = KBB // 2                        # 8 x DMA tiles
NXCOL = 2 * (NXT - 2) + 8 * 2         # pm columns used by x tiles
NCOL = NXCOL + 8                      # + 8 ws half-tiles

F32 = mybir.dt.float32
BF16 = mybir.dt.bfloat16
FP8 = mybir.dt.float8e4
AX = mybir.AxisListType.X
OP = mybir.AluOpType
ACTF = mybir.ActivationFunctionType

_cache: dict = {}


def _emit(tc, nc, xt_d, wt_d, ws_d, b_d, out_d):
    with tc.tile_pool(name="stat", bufs=1) as stat, \
         tc.tile_pool(name="x8p", bufs=1) as x8p, \
         tc.tile_pool(name="dram", bufs=1, space="DRAM") as dram:

        pm = stat.tile([128, NCOL], F32)
        bias_sb = stat.tile([128, NB], F32)

        # warmup collective: absorbs the CC stack's one-time setup during
        # phase A so the real amax AllGather isn't paying for it
        cin_w = dram.tile([1, 1], F32)
        cout_w = dram.tile([N_CORES, 1], F32)
        warm = stat.tile([1, 1], F32)
        nc.vector.memset(warm[:], 0.0)
        nc.scalar.dma_start(cin_w[:], warm[:])
        nc.gpsimd.collective_compute(
            "AllGather", OP.bypass,
            replica_groups=[list(range(N_CORES))],
            ins=[cin_w.opt()], outs=[cout_w.opt()],
        )

        # resident quantized x, pair-interleaved:
        # x8[p, kbb, t*2+i] = q_x * x[token t, k=256*kbb+128*i+p]
        x8 = x8p.tile([128, KBB, 2 * TC], FP8)

        qsc = stat.tile([128, 2], F32)
        scomb = stat.tile([128, 1], F32)

        # ---- phase A: stream x shard (kept resident) + w slice, absmax
        # reduce.  Only SP and Activation have hardware DGE queues (gpsimd
        # DMA is a ~80GB/s software queue), so all bulk traffic alternates
        # between those two.
        with tc.tile_pool(name="xap", bufs=NXT) as xap, \
             tc.tile_pool(name="wsp", bufs=2) as wsp:
            # ws groups stream first so the amax-critical tail is x tiles,
            # whose final reduces are split into 512-column pieces
            # every 2MB tile is split into two 1MB half-DMAs, one per queue,
            # so both queues carry identical loads and drain simultaneously
            ws_tiles = []
            for g in range(4):
                wsl = wsp.tile([128, 2, 2048], F32, name="wsl", tag="wsl")
                for h, eng2 in ((0, nc.sync), (1, nc.scalar)):
                    eng2.dma_start(wsl[:, h, :], ws_d[g, :, h * 2048:(h + 1) * 2048])
                    c = NXCOL + g * 2 + h
                    nc.vector.tensor_reduce(pm[:, c:c + 1], wsl[:, h, :],
                                            AX, OP.max,
                                            apply_absolute_value=True)
                ws_tiles.append(wsl)
            xa_tiles = []
            col = 0
            for xt in range(NXT):
                xa = xap.tile([128, 2, 2 * TC], F32, name=f"xa{xt}", tag="xa")
                for i, eng in ((0, nc.sync), (1, nc.scalar)):
                    eng.dma_start(xa[:, i, :], xt_d[xt, :, i, :])
                flat = xa.rearrange("p a b -> p (a b)")
                nsub = 2 if xt < NXT - 2 else 8
                for q in range(nsub):
                    w = (2 * 2 * TC) // nsub
                    nc.vector.tensor_reduce(
                        pm[:, col:col + 1], flat[:, q * w:(q + 1) * w],
                        AX, OP.max, apply_absolute_value=True)
                    col += 1
                xa_tiles.append(xa)

            am = stat.tile([128, 2], F32)
            nc.vector.tensor_reduce(am[:, 0:1], pm[:, 0:NXCOL], AX, OP.max)
            nc.vector.tensor_reduce(am[:, 1:2], pm[:, NXCOL:NCOL], AX, OP.max)
            amr = stat.tile([128, 2], F32)
            nc.gpsimd.partition_all_reduce(amr[:], am[:], channels=128,
                                           reduce_op=bass_isa.ReduceOp.max)

            # ---- cross-core AllGather of (amax_x, amax_w), then local max.
            # AllGather has ~2x lower latency than AllReduce on this fabric.
            cin = dram.tile([1, 2], F32)
            cout = dram.tile([N_CORES, 2], F32)
            nc.scalar.dma_start(cin[:], amr[0:1, :])
            nc.gpsimd.collective_compute(
                "AllGather", OP.bypass,
                replica_groups=[list(range(N_CORES))],
                ins=[cin.opt()], outs=[cout.opt()],
            )

            # the ~18us collective window: DMA is idle, so prefetch the full
            # j=0 and j=1 weight panels through the freed ws slots + bias
            with tc.tile_pool(name="w8p", bufs=5) as w8p, \
                 tc.tile_pool(name="psp", bufs=4, space="PSUM") as psp, \
                 tc.tile_pool(name="obp", bufs=2) as obp:
                wf_pre = []
                for pj, peng in ((0, nc.sync), (1, nc.scalar)):
                    wfj = wsp.tile([128, 2, KB // 2, 128], F32,
                                   name=f"wf{pj}", tag="wsl")
                    for ph in range(2):
                        peng.dma_start(wfj[:, ph], wt_d[pj, ph])
                    wf_pre.append(wfj)
                nc.scalar.dma_start(bias_sb[:], b_d[:])

                # read the gathered [8,2] maxes into ONE partition and do the
                # global max as a strided DVE reduce (no gpsimd round-trip)
                g1 = stat.tile([1, N_CORES * 2], F32)
                nc.scalar.dma_start(g1[:], cout[:])
                g1v = g1.rearrange("p (g c) -> p c g", c=2)
                g1r = stat.tile([1, 2], F32)
                nc.vector.tensor_reduce(g1r[:], g1v, AX, OP.max)
                gam = stat.tile([128, 2], F32)
                nc.gpsimd.partition_broadcast(gam[:], g1r[:], channels=128)

                # scales: rec ~= 1/amax (reciprocal + 1 Newton step),
                # q = 224*rec, scomb = amax_x*amax_w/50176 (= s_i*s_w*4)
                rec = stat.tile([128, 2], F32)
                tmp = stat.tile([128, 2], F32)
                nc.vector.reciprocal(rec[:], gam[:])
                nc.vector.tensor_tensor(tmp[:], gam[:], rec[:], OP.mult)
                nc.vector.tensor_scalar(tmp[:], tmp[:], -1.0, 2.0, OP.mult,
                                        OP.add)
                nc.vector.tensor_tensor(rec[:], rec[:], tmp[:], OP.mult)
                nc.vector.tensor_scalar_mul(qsc[:], rec[:], 224.0)
                nc.vector.tensor_tensor(scomb[:], gam[:, 0:1], gam[:, 1:2],
                                        OP.mult)
                nc.vector.tensor_scalar_mul(scomb[:], scomb[:], 1.0 / 50176.0)

                # ---- phase B: quantize resident x (frees xa slots, kbb
                # order).  Production is JIT-scheduled against the PE's
                # consumption order for j=0/j=1: DVE takes all tt0 pieces +
                # odd tt1 pieces + the j=1 weight halves; ACT interleaves
                # the j=0 weight chunks with even tt1 pieces.  (gpsimd
                # tensor ops are software-emulated — never use it for bulk.)
                # weight-fp8 tiles are half-j [128, 16, 128] so a streamed
                # panel's quantize only waits for the PE to retire half a
                # j-block, not a whole one
                w8_0 = [w8p.tile([128, KB // 2, 128], FP8, name=f"w8_0h{h}",
                                 tag="w8") for h in range(2)]
                w8_1 = [w8p.tile([128, KB // 2, 128], FP8, name=f"w8_1h{h}",
                                 tag="w8") for h in range(2)]

                def w80_chunk(c):
                    h, q = c // 4, c % 4
                    nc.scalar.activation(
                        w8_0[h][:, q * 4:(q + 1) * 4, :],
                        wf_pre[0][:, h, q * 4:(q + 1) * 4, :],
                        ACTF.Copy, scale=qsc[:, 1:2])

                w80_chunk(0)
                for kbb in range(KBB):
                    xaf = xa_tiles[kbb // 2][:, kbb % 2, :]
                    nc.vector.tensor_scalar_mul(x8[:, kbb, 0:TC],
                                                xaf[:, 0:TC], qsc[:, 0:1])
                    if kbb % 2 == 0:
                        nc.scalar.activation(x8[:, kbb, TC:2 * TC],
                                             xaf[:, TC:2 * TC],
                                             ACTF.Copy, scale=qsc[:, 0:1])
                        if kbb // 2 + 1 < 8:
                            w80_chunk(kbb // 2 + 1)
                    else:
                        nc.vector.tensor_scalar_mul(x8[:, kbb, TC:2 * TC],
                                                    xaf[:, TC:2 * TC],
                                                    qsc[:, 0:1])
                    if kbb in (10, 13):
                        h = 0 if kbb == 10 else 1
                        nc.vector.tensor_scalar_mul(
                            w8_1[h][:], wf_pre[1][:, h], qsc[:, 1:2])

                _matmul_phase(tc, nc, xap, w8p, psp, obp, wt_d, out_d,
                              x8, qsc, scomb, bias_sb, w8_0, w8_1)


def _matmul_phase(tc, nc, xap, w8p, psp, obp, wt_d, out_d, x8, qsc,
                  scomb, bias_sb, w8_0, w8_1):
    for j in range(NB):
        if j == 0:
            w8 = w8_0
        elif j == 1:
            w8 = w8_1
        else:
            w8 = []
            for h in range(2):
                wfh = xap.tile([128, KB // 2, 128], F32,
                               name=f"wf{j}_{h}", tag="xa")
                # all W panels on the sync queue: the SP engine has no
                # compute duties, so descriptors are issued the moment
                # the pool slot frees and the queue never starves the PE
                nc.sync.dma_start(wfh[:], wt_d[j, h])
                w8h = w8p.tile([128, KB // 2, 128], FP8,
                               name=f"w8_{j}h{h}", tag="w8")
                nc.vector.tensor_scalar_mul(w8h[:], wfh[:], qsc[:, 1:2])
                w8.append(w8h)
        pts = [psp.tile([128, TFREE], F32, name=f"pt{tt}", tag=f"pt{tt}")
               for tt in range(TT)]
        for kbb in range(KBB):
            h = kbb // (KBB // 2)
            kl = kbb - h * (KBB // 2)
            lhs = w8[h][:, 2 * kl:2 * kl + 2, :]
            for tt in range(TT):
                rhs = x8[:, kbb, tt * 2 * TFREE:(tt + 1) * 2 * TFREE]
                rhs = rhs.rearrange("p (t two) -> p two t", two=2)
                nc.tensor.matmul(pts[tt][:], lhs, rhs,
                                 start=(kbb == 0), stop=(kbb == KBB - 1),
                                 perf_mode=mybir.MatmulPerfMode.DoubleRow)
        ob = obp.tile([128, TT * TFREE], BF16, name="ob", tag="ob")
        for tt in range(TT):
            sl = slice(tt * TFREE, (tt + 1) * TFREE)
            nc.scalar.activation(ob[:, sl], pts[tt][:], ACTF.Identity,
                                 bias=bias_sb[:, j:j + 1], scale=scomb[:])
            if j == NB - 1:
                # flush the final block per-half so the kernel tail isn't
                # serialized behind the second epilogue
                nc.scalar.dma_start(out_d[j][:, sl], ob[:, sl])
        if j != NB - 1:
            # one contiguous 256KB write per j, issued by ACT right after
            # its own epilogues (host de-blocks [j, n, t] -> [t, N])
            nc.scalar.dma_start(out_d[j], ob[:])


def _build():
    nc = bacc.Bacc("TRN2", target_bir_lowering=False, debug=False,
                   enable_asserts=False, num_devices=N_CORES)
    xt_d = nc.dram_tensor("xt", [KBB // 2, 128, 2, 2 * TC], F32,
                          kind="ExternalInput").ap()
    wt_d = nc.dram_tensor("wt", [NB, 2, 128, KB // 2, 128], F32,
                          kind="ExternalInput").ap()
    ws_d = nc.dram_tensor("ws", [WS // 128, 128, K], F32,
                          kind="ExternalInput").ap()
    b_d = nc.dram_tensor("bias", [128, NB], F32, kind="ExternalInput").ap()
    out_d = nc.dram_tensor("out", [NB, 128, TT * TFREE], BF16,
                           kind="ExternalOutput").ap()
    with tile.TileContext(nc) as tc:
        _emit(tc, nc, xt_d, wt_d, ws_d, b_d, out_d)
    nc.compile()
    return nc


def _prepare_inputs(x, weight, bias):
    x = np.ascontiguousarray(np.asarray(x, dtype=np.float32))
    weight = np.ascontiguousarray(np.asarray(weight, dtype=np.float32))
    bias = np.ascontiguousarray(np.asarray(bias, dtype=np.float32))

    x2 = x.reshape(T, K)
    # weight [N, K] -> blocked W^T with contiguous 1MB half-panels:
    # wt[j, h, p, kbh, n] = weight[j*128+n, (h*16+kbh)*128+p]
    wt = np.ascontiguousarray(
        weight.reshape(NB, 128, KB, 128).transpose(0, 3, 2, 1)
        .reshape(NB, 128, 2, KB // 2, 128).transpose(0, 2, 1, 3, 4))
    bias_dev = np.ascontiguousarray(bias.reshape(NB, 128).T)  # [128, NB]

    in_maps = []
    for c in range(N_CORES):
        xs = x2[c * TC:(c + 1) * TC, :]                  # [TC, K]
        # -> [kbb, p, t, i] with k = kbb*256 + i*128 + p, flattened (t,i),
        # then paired into 2MB DMA tiles [xt, p, kbb%2, t*2+i]
        xdev = np.ascontiguousarray(
            xs.reshape(TC, KBB, 2, 128).transpose(1, 3, 0, 2)
        ).reshape(KBB // 2, 2, 128, 2 * TC).transpose(0, 2, 1, 3)
        in_maps.append({
            "xt": np.ascontiguousarray(xdev),
            "wt": wt,
            "ws": np.ascontiguousarray(
                weight[c * WS:(c + 1) * WS, :]).reshape(WS // 128, 128, K),
            "bias": bias_dev,
        })
    return in_maps


def _run(x, weight, bias, trace=False):
    if "nc" not in _cache:
        _cache["nc"] = _build()
    nc = _cache["nc"]
    in_maps = _prepare_inputs(x, weight, bias)
    res = bass_utils.run_bass_kernel_spmd(
        nc, in_maps, core_ids=list(range(N_CORES)), trace=trace)
    out = np.empty((T, N), dtype=np.float32)
    for c in range(N_CORES):
        buf = res.results[c]["out"].astype(np.float32)  # [NB, 128, TC]
        out[c * TC:(c + 1) * TC, :] = buf.transpose(2, 0, 1).reshape(TC, N)
    return out.reshape(XB, XS, N), res


def kernel(x, weight, bias):
    out, _ = _run(x, weight, bias, trace=False)
    return out


# revision 38
# speedup vs baseline: 1.1166x; 1.1166x over previous
"""FP8 per-tensor dynamic-quantized Linear on 8 TRN2 NeuronCores.

Computes reference semantics of:
    x2 = x.reshape(-1, 4096)
    x_fp8, s_i = quantize_e4m3fn(x2)      # per-tensor amax -> scale
    w_fp8, s_w = quantize_e4m3fn(weight)
    out = (x_fp8.f32 @ w_fp8.f32.T) * (s_i * s_w) + bias

Sharding: token-parallel. Each core owns 1024 tokens of x (stored k-major,
DoubleRow pair-interleaved), the full weight (blocked layout), and a distinct
1/8 slice of the weight rows for the distributed amax pass. Global per-tensor
amax for x and w = local absmax reduce + one 8-core AllGather(max) of 2
floats.

TRN fp8_e4m3 saturates at +-240 (OCP e4m3fn goes to 448), so the device
quantizes with multiplier 224/amax == (448/amax)/2 exactly (power-of-two =>
bit-identical RNE mantissa rounding vs the reference) and the matmul output
is rescaled by (amax_x*amax_w)/50176 = 4*s_i*s_w to compensate the halvings.

Matmul runs in fp8 DoubleRow mode: stationary W tile [128,2,128] plane-major,
moving x tile pair-interleaved ([p, t, 2] memory viewed as [p, 2, t]) which
the PE streams at 2 fp8/cycle, accumulating 16 k-blocks into PSUM. Epilogue =
single ScalarE activation (scale + per-partition bias) emitting bf16; output
is written as contiguous [j, n, t] blocks and de-blocked (cast + transpose)
on the host. bf16 keeps the phase-C DMA demand (~2.2MB per 8.5us j-slot)
under the per-core HBM share so the PE never starves.

Scheduling notes (measured on the axon trn2 cores): the PE is power-throttled
to 81.25% duty (264ns per 512-row DR matmul; ~270us floor for the 1024
matmuls). Only SP and Activation have hardware DGE queues — gpsimd DMA is a
~80GB/s software queue and gpsimd tensor ops are software-emulated, so all
bulk DMA rides sync/scalar (W panels sync-only: the SP engine has no compute
duties, so the weight stream is never issued late) and all bulk elementwise
rides DVE/ACT. A warmup AllGather issued at t=0 absorbs the CC stack's
one-time setup; the real 8-byte amax AllGather costs ~18-25us. j=0/j=1
weight panels are prefetched during the collective window and their fp8
quantize is JIT-interleaved with the x8 quantize against the PE's
consumption order.
"""

import numpy as np

import concourse.bass as bass
import concourse.mybir as mybir
import concourse.tile as tile
from concourse import bacc, bass_isa, bass_utils

N_CORES = 8
XB, XS, K = 4, 2048, 4096   # x: [4, 2048, 4096]
N = 4096                    # weight: [N, K]
T = XB * XS                 # 8192 tokens
TC = T // N_CORES           # 1024 tokens per core
NB = N // 128               # 32 output-feature blocks
KB = K // 128               # 32 k subtiles of 128
KBB = K // 256              # 16 DoubleRow k blocks
TFREE = 512
TT = TC // TFREE            # 2 token tiles per core
WS = N // N_CORES           # 512 weight rows per core for amax

# x arrives as 8 2MB tiles (2 kbb each); each is absmax-reduced in halves,
# the last two (one per queue) in eighths so the final reduce trails the
# final DMA by ~0.6us
NXT = KBB // 2                        # 8 x DMA tiles
NXCOL = 2 * (NXT - 2) + 8 * 2         # pm columns used by x tiles
NCOL = NXCOL + 8                      # + 8 ws half-tiles

F32 = mybir.dt.float32
BF16 = mybir.dt.bfloat16
FP8 = mybir.dt.float8e4
AX = mybir.AxisListType.X
OP = mybir.AluOpType
ACTF = mybir.ActivationFunctionType

_cache: dict = {}


def _emit(tc, nc, xt_d, wt_d, ws_d, b_d, out_d):
    with tc.tile_pool(name="stat", bufs=1) as stat, \
         tc.tile_pool(name="x8p", bufs=1) as x8p, \
         tc.tile_pool(name="dram", bufs=1, space="DRAM") as dram:

        pm = stat.tile([128, NCOL], F32)
        bias_sb = stat.tile([128, NB], F32)

        # warmup collective: absorbs the CC stack's one-time setup during
        # phase A so the real amax AllGather isn't paying for it
        cin_w = dram.tile([1, 1], F32)
        cout_w = dram.tile([N_CORES, 1], F32)
        warm = stat.tile([1, 1], F32)
        nc.vector.memset(warm[:], 0.0)
        nc.scalar.dma_start(cin_w[:], warm[:])
        nc.gpsimd.collective_compute(
            "AllGather", OP.bypass,
            replica_groups=[list(range(N_CORES))],
            ins=[cin_w.opt()], outs=[cout_w.opt()],
        )

        # resident quantized x, pair-interleaved:
        # x8[p, kbb, t*2+i] = q_x * x[token t, k=256*kbb+128*i+p]
        x8 = x8p.tile([128, KBB, 2 * TC], FP8)

        qsc = stat.tile([128, 2], F32)
        scomb = stat.tile([128, 1], F32)

        # ---- phase A: stream x shard (kept resident) + w slice, absmax
        # reduce.  Only SP and Activation have hardware DGE queues (gpsimd
        # DMA is a ~80GB/s software queue), so all bulk traffic alternates
        # between those two.
        with tc.tile_pool(name="xap", bufs=NXT) as xap, \
             tc.tile_pool(name="wsp", bufs=2) as wsp:
            # ws groups stream first so the amax-critical tail is x tiles,
            # whose final reduces are split into 512-column pieces
            # every 2MB tile is split into two 1MB half-DMAs, one per queue,
            # so both queues carry identical loads and drain simultaneously
            ws_tiles = []
            for g in range(4):
                wsl = wsp.tile([128, 2, 2048], F32, name="wsl", tag="wsl")
                for h, eng2 in ((0, nc.sync), (1, nc.scalar)):
                    eng2.dma_start(wsl[:, h, :], ws_d[g, :, h * 2048:(h + 1) * 2048])
                    c = NXCOL + g * 2 + h
                    nc.vector.tensor_reduce(pm[:, c:c + 1], wsl[:, h, :],
                                            AX, OP.max,
                                            apply_absolute_value=True)
                ws_tiles.append(wsl)
            xa_tiles = []
            col = 0
            for xt in range(NXT):
                xa = xap.tile([128, 2, 2 * TC], F32, name=f"xa{xt}", tag="xa")
                for i, eng in ((0, nc.sync), (1, nc.scalar)):
                    eng.dma_start(xa[:, i, :], xt_d[xt, :, i, :])
                flat = xa.rearrange("p a b -> p (a b)")
                nsub = 2 if xt < NXT - 2 else 8
                for q in range(nsub):
                    w = (2 * 2 * TC) // nsub
                    nc.vector.tensor_reduce(
                        pm[:, col:col + 1], flat[:, q * w:(q + 1) * w],
                        AX, OP.max, apply_absolute_value=True)
                    col += 1
                xa_tiles.append(xa)

            am = stat.tile([128, 2], F32)
            nc.vector.tensor_reduce(am[:, 0:1], pm[:, 0:NXCOL], AX, OP.max)
            nc.vector.tensor_reduce(am[:, 1:2], pm[:, NXCOL:NCOL], AX, OP.max)
            amr = stat.tile([128, 2], F32)
            nc.gpsimd.partition_all_reduce(amr[:], am[:], channels=128,
                                           reduce_op=bass_isa.ReduceOp.max)

            # ---- cross-core AllGather of (amax_x, amax_w), then local max.
            # AllGather has ~2x lower latency than AllReduce on this fabric.
            cin = dram.tile([1, 2], F32)
            cout = dram.tile([N_CORES, 2], F32)
            nc.scalar.dma_start(cin[:], amr[0:1, :])
            nc.gpsimd.collective_compute(
                "AllGather", OP.bypass,
                replica_groups=[list(range(N_CORES))],
                ins=[cin.opt()], outs=[cout.opt()],
            )

            # the ~18us collective window: DMA is idle, so prefetch the full
            # j=0 and j=1 weight panels through the freed ws slots + bias
            with tc.tile_pool(name="w8p", bufs=5) as w8p, \
                 tc.tile_pool(name="psp", bufs=4, space="PSUM") as psp, \
                 tc.tile_pool(name="obp", bufs=2) as obp:
                wf_pre = []
                for pj, peng in ((0, nc.sync), (1, nc.scalar)):
                    wfj = wsp.tile([128, 2, KB // 2, 128], F32,
                                   name=f"wf{pj}", tag="wsl")
                    for ph in range(2):
                        peng.dma_start(wfj[:, ph], wt_d[pj, ph])
                    wf_pre.append(wfj)
                nc.scalar.dma_start(bias_sb[:], b_d[:])

                # read the gathered [8,2] maxes into ONE partition and do the
                # global max as a strided DVE reduce (no gpsimd round-trip)
                g1 = stat.tile([1, N_CORES * 2], F32)
                nc.scalar.dma_start(g1[:], cout[:])
                g1v = g1.rearrange("p (g c) -> p c g", c=2)
                g1r = stat.tile([1, 2], F32)
                nc.vector.tensor_reduce(g1r[:], g1v, AX, OP.max)
                gam = stat.tile([128, 2], F32)
                nc.gpsimd.partition_broadcast(gam[:], g1r[:], channels=128)

                # scales: rec ~= 1/amax (reciprocal + 1 Newton step),
                # q = 224*rec, scomb = amax_x*amax_w/50176 (= s_i*s_w*4)
                rec = stat.tile([128, 2], F32)
                tmp = stat.tile([128, 2], F32)
                nc.vector.reciprocal(rec[:], gam[:])
                nc.vector.tensor_tensor(tmp[:], gam[:], rec[:], OP.mult)
                nc.vector.tensor_scalar(tmp[:], tmp[:], -1.0, 2.0, OP.mult,
                                        OP.add)
                nc.vector.tensor_tensor(rec[:], rec[:], tmp[:], OP.mult)
                nc.vector.tensor_scalar_mul(qsc[:], rec[:], 224.0)
                nc.vector.tensor_tensor(scomb[:], gam[:, 0:1], gam[:, 1:2],
                                        OP.mult)
                nc.vector.tensor_scalar_mul(scomb[:], scomb[:], 1.0 / 50176.0)

                # ---- phase B: quantize resident x (frees xa slots, kbb
                # order).  Production is JIT-scheduled against the PE's
                # consumption order for j=0/j=1: DVE takes all tt0 pieces +
                # odd tt1 pieces + the j=1 weight halves; ACT interleaves
                # the j=0 weight chunks with even tt1 pieces.  (gpsimd
                # tensor ops are software-emulated — never use it for bulk.)
                # weight-fp8 tiles are half-j [128, 16, 128] so a streamed
                # panel's quantize only waits for the PE to retire half a
                # j-block, not a whole one
                w8_0 = [w8p.tile([128, KB // 2, 128], FP8, name=f"w8_0h{h}",
                                 tag="w8") for h in range(2)]
                w8_1 = [w8p.tile([128, KB // 2, 128], FP8, name=f"w8_1h{h}",
                                 tag="w8") for h in range(2)]

                def w80_chunk(c):
                    h, q = c // 4, c % 4
                    nc.scalar.activation(
                        w8_0[h][:, q * 4:(q + 1) * 4, :],
                        wf_pre[0][:, h, q * 4:(q + 1) * 4, :],
                        ACTF.Copy, scale=qsc[:, 1:2])

                w80_chunk(0)
                for kbb in range(KBB):
                    xaf = xa_tiles[kbb // 2][:, kbb % 2, :]
                    nc.vector.tensor_scalar_mul(x8[:, kbb, 0:TC],
                                                xaf[:, 0:TC], qsc[:, 0:1])
                    if kbb % 2 == 0:
                        nc.scalar.activation(x8[:, kbb, TC:2 * TC],
                                             xaf[:, TC:2 * TC],
                                             ACTF.Copy, scale=qsc[:, 0:1])
                        if kbb // 2 + 1 < 8:
                            w80_chunk(kbb // 2 + 1)
                    else:
                        nc.vector.tensor_scalar_mul(x8[:, kbb, TC:2 * TC],
                                                    xaf[:, TC:2 * TC],
                                                    qsc[:, 0:1])
                    # j=1 weight halves on ACT (DVE already carries 24 x8
                    # pieces; this balances the two post-scales chains)
                    if kbb in (10, 14):
                        h = 0 if kbb == 10 else 1
                        nc.scalar.activation(w8_1[h][:], wf_pre[1][:, h],
                                             ACTF.Copy, scale=qsc[:, 1:2])

                _matmul_phase(tc, nc, xap, w8p, psp, obp, wt_d, out_d,
                              x8, qsc, scomb, bias_sb, w8_0, w8_1)


def _matmul_phase(tc, nc, xap, w8p, psp, obp, wt_d, out_d, x8, qsc,
                  scomb, bias_sb, w8_0, w8_1):
    for j in range(NB):
        if j == 0:
            w8 = w8_0
        elif j == 1:
            w8 = w8_1
        else:
            w8 = []
            for h in range(2):
                wfh = xap.tile([128, KB // 2, 128], F32,
                               name=f"wf{j}_{h}", tag="xa")
                # all W panels on the sync queue: the SP engine has no
                # compute duties, so descriptors are issued the moment
                # the pool slot frees and the queue never starves the PE
                nc.sync.dma_start(wfh[:], wt_d[j, h])
                w8h = w8p.tile([128, KB // 2, 128], FP8,
                               name=f"w8_{j}h{h}", tag="w8")
                nc.vector.tensor_scalar_mul(w8h[:], wfh[:], qsc[:, 1:2])
                w8.append(w8h)
        pts = [psp.tile([128, TFREE], F32, name=f"pt{tt}", tag=f"pt{tt}")
               for tt in range(TT)]
        for kbb in range(KBB):
            h = kbb // (KBB // 2)
            kl = kbb - h * (KBB // 2)
            lhs = w8[h][:, 2 * kl:2 * kl + 2, :]
            for tt in range(TT):
                rhs = x8[:, kbb, tt * 2 * TFREE:(tt + 1) * 2 * TFREE]
                rhs = rhs.rearrange("p (t two) -> p two t", two=2)
                nc.tensor.matmul(pts[tt][:], lhs, rhs,
                                 start=(kbb == 0), stop=(kbb == KBB - 1),
                                 perf_mode=mybir.MatmulPerfMode.DoubleRow)
        ob = obp.tile([128, TT * TFREE], BF16, name="ob", tag="ob")
        for tt in range(TT):
            sl = slice(tt * TFREE, (tt + 1) * TFREE)
            nc.scalar.activation(ob[:, sl], pts[tt][:], ACTF.Identity,
                                 bias=bias_sb[:, j:j + 1], scale=scomb[:])
            if j == NB - 1:
                # flush the final block per-half so the kernel tail isn't
                # serialized behind the second epilogue
                nc.scalar.dma_start(out_d[j][:, sl], ob[:, sl])
        if j != NB - 1:
            # one contiguous 256KB write per j, issued by ACT right after
            # its own epilogues (host de-blocks [j, n, t] -> [t, N])
            nc.scalar.dma_start(out_d[j], ob[:])


def _build():
    nc = bacc.Bacc("TRN2", target_bir_lowering=False, debug=False,
                   enable_asserts=False, num_devices=N_CORES)
    xt_d = nc.dram_tensor("xt", [KBB // 2, 128, 2, 2 * TC], F32,
                          kind="ExternalInput").ap()
    wt_d = nc.dram_tensor("wt", [NB, 2, 128, KB // 2, 128], F32,
                          kind="ExternalInput").ap()
    ws_d = nc.dram_tensor("ws", [WS // 128, 128, K], F32,
                          kind="ExternalInput").ap()
    b_d = nc.dram_tensor("bias", [128, NB], F32, kind="ExternalInput").ap()
    out_d = nc.dram_tensor("out", [NB, 128, TT * TFREE], BF16,
                           kind="ExternalOutput").ap()
    with tile.TileContext(nc) as tc:
        _emit(tc, nc, xt_d, wt_d, ws_d, b_d, out_d)
    nc.compile()
    return nc


def _prepare_inputs(x, weight, bias):
    x = np.ascontiguousarray(np.asarray(x, dtype=np.float32))
    weight = np.ascontiguousarray(np.asarray(weight, dtype=np.float32))
    bias = np.ascontiguousarray(np.asarray(bias, dtype=np.float32))

    x2 = x.reshape(T, K)
    # weight [N, K] -> blocked W^T with contiguous 1MB half-panels:
    # wt[j, h, p, kbh, n] = weight[j*128+n, (h*16+kbh)*128+p]
    wt = np.ascontiguousarray(
        weight.reshape(NB, 128, KB, 128).transpose(0, 3, 2, 1)
        .reshape(NB, 128, 2, KB // 2, 128).transpose(0, 2, 1, 3, 4))
    bias_dev = np.ascontiguousarray(bias.reshape(NB, 128).T)  # [128, NB]

    in_maps = []
    for c in range(N_CORES):
        xs = x2[c * TC:(c + 1) * TC, :]                  # [TC, K]
        # -> [kbb, p, t, i] with k = kbb*256 + i*128 + p, flattened (t,i),
        # then paired into 2MB DMA tiles [xt, p, kbb%2, t*2+i]
        xdev = np.ascontiguousarray(
            xs.reshape(TC, KBB, 2, 128).transpose(1, 3, 0, 2)
        ).reshape(KBB // 2, 2, 128, 2 * TC).transpose(0, 2, 1, 3)
        in_maps.append({
            "xt": np.ascontiguousarray(xdev),
            "wt": wt,
            "ws": np.ascontiguousarray(
                weight[c * WS:(c + 1) * WS, :]).reshape(WS // 128, 128, K),
            "bias": bias_dev,
        })
    return in_maps


def _run(x, weight, bias, trace=False):
    if "nc" not in _cache:
        _cache["nc"] = _build()
    nc = _cache["nc"]
    in_maps = _prepare_inputs(x, weight, bias)
    res = bass_utils.run_bass_kernel_spmd(
        nc, in_maps, core_ids=list(range(N_CORES)), trace=trace)
    out = np.empty((T, N), dtype=np.float32)
    for c in range(N_CORES):
        buf = res.results[c]["out"].astype(np.float32)  # [NB, 128, TC]
        out[c * TC:(c + 1) * TC, :] = buf.transpose(2, 0, 1).reshape(TC, N)
    return out.reshape(XB, XS, N), res


def kernel(x, weight, bias):
    out, _ = _run(x, weight, bias, trace=False)
    return out
